# revision 13
# baseline (speedup 1.0000x reference)
"""Trainium2 Bass kernel for the ConsistencyLoss problem (v5).

Inputs: semantic_pred (B,N) int32, instance_masks (B,M,N) f32, depth (B,N) f32
with B=16, M=32, N=65536 (H=W=256), C=27 classes. Outputs the scalar tuple
(l_uniform, l_boundary, l_dbc, total).

Sharding: pure data-parallel over batch; 2 batches per core on 8 cores. Each
core emits 12 partial sums (6 per batch); the host combines the 4 scalars.

v5 layout (DMA-saturation rewrite of the v4 baseline):
  * masks stream as ONE [128=(g,m), 4096] f32 HWDGE DMA per supertile (2 MB,
    8 per core, all on the sync ring) instead of 4 quarter DMAs from three
    engines. f32->bf16 casts are split ACT/[gpsimd] at a 64-px-aligned column
    CA so each transpose half depends on exactly one cast.
  * DVE pair-transpose (bf16 pairs bitcast to u32, 32x32 block semantics) as
    before; one-hot built per supertile from a transposed sem tile against a
    materialized iota replica (built once from a [128,54] iota).
  * histogram matmuls: 64 per supertile, stationary [128,64]=(m,e) slices,
    moving [128,54] one-hot; PSUM accumulates the interleaved histogram.
  * boundary/depth losses use a row-pair layout [p, (b, r2, w)] with rows
    (2p, 2p+1): y-gradients for odd rows stay in-tile; even rows use ONE
    shifted [127, (b,w)] HBM DMA per tensor (odd source rows) + a tiny
    partition-0 fixup. All small DMAs ride the scalar ring.
  * 12 partials: per batch {S_ib, S_nb, S_nbib, S_nbdb, S_nbs2, entropy};
    host combines (nbv = nbdb + 3*nbs2 since db^2 = s2).
"""

import os

os.environ.setdefault("MYCRO_LOCAL_CACHE", "1")

import numpy as np
from contextlib import ExitStack

B, M, N, C = 16, 32, 65536, 27
H = W = 256
NCORES = 8
BPC = B // NCORES          # batches per core
ST = 4                     # supertiles per batch (16384 px each)
G = 4                      # partition groups: partition = 32*g + m
FS = 4096                  # pixels per partition per supertile
KB = 64                    # 64-px stationary blocks per supertile
CA = 2560                  # ACT-cast / transpose-A split (64-aligned)
KA = CA // 64              # kb blocks in half A
NSTAT = 12                 # 6 partial sums x 2 batches

LAST_EXEC_NS = None
LAST_TRACE = None
LAST_STATS = None

_compiled = None


def _build():
    import concourse.tile as tile
    from concourse import bacc, mybir

    f32 = mybir.dt.float32
    i32 = mybir.dt.int32
    bf16 = mybir.dt.bfloat16
    OP = mybir.AluOpType
    AX = mybir.AxisListType
    AF = mybir.ActivationFunctionType

    nc = bacc.Bacc("TRN2", target_bir_lowering=False, debug=False,
                   enable_asserts=False, num_swdge_queues=4)
    sem_d = nc.dram_tensor("sem", [BPC, N], i32, kind="ExternalInput")
    masks_d = nc.dram_tensor("masks", [BPC, M, N], f32, kind="ExternalInput")
    depth_d = nc.dram_tensor("depth", [BPC, N], f32, kind="ExternalInput")
    out_d = nc.dram_tensor("partials", [NSTAT], f32, kind="ExternalOutput")

    with tile.TileContext(nc) as tc, ExitStack() as ctx:
        pconst = ctx.enter_context(tc.tile_pool(name="const", bufs=1))
        pin = ctx.enter_context(tc.tile_pool(name="maskin", bufs=3))
        pinb = ctx.enter_context(tc.tile_pool(name="maskinb", bufs=3))
        ptr = ctx.enter_context(tc.tile_pool(name="maskT", bufs=3))
        poh = ctx.enter_context(tc.tile_pool(name="ohp", bufs=2))
        psem = ctx.enter_context(tc.tile_pool(name="semp", bufs=1))
        pbnd = ctx.enter_context(tc.tile_pool(name="bnd", bufs=1))
        pscr = ctx.enter_context(tc.tile_pool(name="scr", bufs=7))
        psm = ctx.enter_context(tc.tile_pool(name="small", bufs=2))
        pps = ctx.enter_context(tc.tile_pool(name="psum", bufs=1, space="PSUM"))

        do_hist = not bool(int(os.environ.get("KERNEL_SKIP_HIST", "0")))

        # ---------------- constants ----------------
        iota54_i = pconst.tile([128, 2 * C], i32, tag="io54i")
        nc.gpsimd.iota(iota54_i[:], pattern=[[1, C], [0, 2]], base=0,
                       channel_multiplier=0)
        iota54 = pconst.tile([128, 2 * C], bf16, tag="io54")
        nc.scalar.copy(iota54[:], iota54_i[:])
        iorep = pconst.tile([128, 2 * C * KB], bf16, tag="iorep")
        nc.vector.tensor_copy(
            iorep[:].rearrange("p (c k e) -> p c k e", c=C, k=KB),
            iota54[:].rearrange("p (c e) -> p c e", c=C)
                .unsqueeze(2).broadcast_to([128, C, KB, 2]))

        stats = pconst.tile([128, NSTAT], f32, tag="stats")
        nc.gpsimd.memset(stats[:], 0.0)
        ones = pconst.tile([128, 1], f32, tag="ones")
        nc.gpsimd.memset(ones[:], 1.0)
        bias_ln = pconst.tile([M, 1], f32, tag="bias_ln")
        nc.gpsimd.memset(bias_ln[:], 1e-10)
        bias_sq = pconst.tile([128, 1], f32, tag="bias_sq")
        nc.gpsimd.memset(bias_sq[:], 1e-24)
        zerob = pconst.tile([128, 1], f32, tag="zerob")
        nc.gpsimd.memset(zerob[:], 0.0)
        warm = pconst.tile([128, 1], f32, tag="warm")
        nc.scalar.activation(warm[:], zerob[:], AF.Sqrt, bias=bias_sq[:, 0:1])

        # parity-selection matrices: sele[p, m] = (p == 2m), selo: (p == 2m+1)
        selrow = pconst.tile([64, M], i32, tag="selrow")
        nc.gpsimd.iota(selrow[:], pattern=[[0, M]], base=0, channel_multiplier=1)
        selc2 = pconst.tile([64, M], i32, tag="selc2")
        nc.gpsimd.iota(selc2[:], pattern=[[2, M]], base=0, channel_multiplier=0)
        selc2p = pconst.tile([64, M], i32, tag="selc2p")
        nc.gpsimd.iota(selc2p[:], pattern=[[2, M]], base=1, channel_multiplier=0)
        sele = pconst.tile([64, M], f32, tag="sele")
        nc.vector.tensor_tensor(sele[:], selrow[:], selc2[:], op=OP.is_equal)
        selo = pconst.tile([64, M], f32, tag="selo")
        nc.vector.tensor_tensor(selo[:], selrow[:], selc2p[:], op=OP.is_equal)

        # -------- sem -> transposed one-hot source (both batches) --------
        # s_in[p, st*128 + cu] = sem[st*16384 + p*128 + cu]; after the bf16
        # pair-transpose, sT[32g+p', st*128 + 64k + 2f' + e] is the pixel
        # st*16384 + g*4096 + (2f'+k)*64 + 2p' + e -- i.e. stationary block
        # kb = 2f'+k of supertile st reads one-hot rows from column block
        # (k, f') of sT.
        sT = []
        for b in range(BPC):
            s_in = psem.tile([128, ST * 128], i32, tag=f"s_in{b}")
            nc.scalar.dma_start(
                s_in[:],
                sem_d.ap()[b].rearrange("(st p cu) -> p st cu",
                                        st=ST, p=128, cu=128))
            s_bf = psem.tile([128, ST * 128], bf16, tag=f"s_bf{b}")
            nc.scalar.copy(s_bf[:], s_in[:])
            s_t = psem.tile([128, ST * 128], bf16, tag=f"s_t{b}")
            nc.vector.transpose(s_t[:].bitcast(i32), s_bf[:].bitcast(i32))
            sT.append(s_t)

        hist_ps = [pps.tile([64, 2 * C], f32, tag=f"hist{b}", name=f"hist{b}")
                   for b in range(BPC)]

        def emit_tile(b, st):
            tin = pin.tile([128, FS], f32, tag="tin", name="tin")
            nc.sync.dma_start(
                tin[:],
                masks_d.ap()[b].rearrange("m (st g f) -> st g m f",
                                          st=ST, g=G)[st])
            tinb = pinb.tile([128, FS], bf16, tag="tinb", name="tinb")
            nc.scalar.copy(tinb[:, 0:CA], tin[:, 0:CA])
            nc.gpsimd.tensor_scalar_add(tinb[:, CA:FS], tin[:, CA:FS], 0.0)
            tT = ptr.tile([128, FS], bf16, tag="tT", name="tT")
            nc.vector.transpose(tT[:, 0:CA].bitcast(i32),
                                tinb[:, 0:CA].bitcast(i32))

            # one-hot: m2[p, (c, kp, f, e)] = (sT[p, st*128+64kp+2f+e] == c)
            m2 = poh.tile([128, 2 * C * KB], bf16, tag="m2", name="m2")
            m2v = m2[:].rearrange("p (c k f e) -> p k c f e", c=C, k=2, f=32)
            iov = iorep[:].rearrange("p (c k f e) -> p k c f e", c=C, k=2, f=32)
            for kp in range(2):
                o = st * 128 + 64 * kp
                nc.vector.tensor_tensor(
                    m2v[:, kp],
                    sT[b][:, o:o + 64]
                        .rearrange("p (f e) -> p f e", f=32)
                        .unsqueeze(1).broadcast_to([128, C, 32, 2]),
                    iov[:, kp],
                    op=OP.is_equal)

            # moving block for stationary kb sits at kf = (kb&1)*32 + (kb>>1)
            mov = m2[:].rearrange("p (c kf e) -> p kf c e", c=C, kf=KB)
            for kb in range(KA):
                nc.tensor.matmul(
                    hist_ps[b][:],
                    tT[:, 64 * kb:64 * (kb + 1)],
                    mov[:, (kb & 1) * 32 + (kb >> 1)],
                    start=(st == 0 and kb == 0),
                    stop=False)
            nc.vector.transpose(tT[:, CA:FS].bitcast(i32),
                                tinb[:, CA:FS].bitcast(i32))
            for kb in range(KA, KB):
                nc.tensor.matmul(
                    hist_ps[b][:],
                    tT[:, 64 * kb:64 * (kb + 1)],
                    mov[:, (kb & 1) * 32 + (kb >> 1)],
                    start=False,
                    stop=(st == ST - 1 and kb == KB - 1))

        def emit_entropy_epilogue(b):
            # hist[m, c] = psum[2m, 2c] + psum[2m+1, 2c+1]
            psum_sb = psm.tile([64, 2 * C], f32, tag="psum_sb", name="psum_sb")
            nc.scalar.copy(psum_sb[:], hist_ps[b][:])
            psv = psum_sb[:].rearrange("p (c e) -> p e c", e=2)
            h_ps = pps.tile([M, C], f32, tag=f"hps{b}", name=f"hps{b}")
            nc.tensor.matmul(h_ps[:], sele[:], psv[:, 0], start=True, stop=False)
            nc.tensor.matmul(h_ps[:], selo[:], psv[:, 1], start=False, stop=True)
            hist = psm.tile([M, C], f32, tag="hist_sb", name="hist_sb")
            nc.scalar.copy(hist[:], h_ps[:])
            ms0 = psm.tile([M, 1], f32, tag="ms0", name="ms0")
            nc.vector.tensor_reduce(ms0[:], hist[:], axis=AX.X, op=OP.add)
            ms = psm.tile([M, 1], f32, tag="ms", name="ms")
            nc.vector.tensor_scalar(ms[:], ms0[:], 1e-6, None, op0=OP.add)
            rec = psm.tile([M, 1], f32, tag="rec", name="rec")
            nc.vector.reciprocal(rec[:], ms[:])
            pr = psm.tile([M, C], f32, tag="pr", name="pr")
            nc.vector.tensor_scalar(pr[:], hist[:], rec[:, 0:1], None, op0=OP.mult)
            ql = psm.tile([M, C], f32, tag="ql", name="ql")
            nc.scalar.activation(ql[:], pr[:], AF.Ln, bias=bias_ln[0:M, 0:1])
            escr = psm.tile([M, C], f32, tag="escr", name="escr")
            nc.vector.tensor_tensor(escr[:], pr[:], ql[:], op=OP.mult)
            ent = psm.tile([M, 1], f32, tag="ent", name="ent")
            nc.vector.tensor_reduce(ent[:], escr[:], axis=AX.X, op=OP.add)
            nc.vector.tensor_scalar(stats[0:M, 6 * b + 5:6 * b + 6], ent[:],
                                    -1.0, None, op0=OP.mult)

        # ---------------- boundary tiles ----------------
        # natural layout [p, (b, r2, w)]: rows (2p + r2); shifted tiles hold
        # row 2p-1 (odd source rows) for the even-row y-gradients.
        semn = pbnd.tile([128, BPC * 2 * W], i32, tag="semn", name="semn")
        m0n = pbnd.tile([128, BPC * 2 * W], f32, tag="m0n", name="m0n")
        dn = pbnd.tile([128, BPC * 2 * W], f32, tag="dn", name="dn")
        sem2 = pbnd.tile([128, BPC * W], i32, tag="sem2", name="sem2")
        m02 = pbnd.tile([128, BPC * W], f32, tag="m02", name="m02")
        d2 = pbnd.tile([128, BPC * W], f32, tag="d2", name="d2")
        nb = pbnd.tile([128, BPC * 2 * W], f32, tag="nb", name="nb")
        ib = pbnd.tile([128, BPC * 2 * W], f32, tag="ib", name="ib")
        s2 = pbnd.tile([128, BPC * 2 * W], f32, tag="s2", name="s2")

        m0_src = masks_d.ap().rearrange("b m n -> m b n")[0]

        def nat_ap(t):
            return t.rearrange("b (p r2 w) -> p b r2 w", p=128, r2=2, w=W)

        def odd_ap(t):
            # odd source rows 1,3,...,253 -> partitions 1..127
            return t[:, W:W + 127 * 2 * W].rearrange(
                "b (p v) -> p b v", p=127)[:, :, 0:W]

        def r4(t):
            return t[:].rearrange("p (b r2 w) -> p b r2 w", b=BPC, r2=2)

        def r3(t):
            return t[:].rearrange("p (b w) -> p b w", b=BPC)

        def blkw(t):
            return t[:].rearrange("p (q w) -> p q w", w=W)

        def emit_boundary_dmas():
            nc.scalar.dma_start(r4(semn), nat_ap(sem_d.ap()))
            nc.scalar.dma_start(r4(m0n), nat_ap(m0_src))
            nc.scalar.dma_start(r4(dn), nat_ap(depth_d.ap()))
            nc.scalar.dma_start(r3(sem2)[1:128], odd_ap(sem_d.ap()))
            nc.scalar.dma_start(r3(m02)[1:128], odd_ap(m0_src))
            nc.scalar.dma_start(r3(d2)[1:128], odd_ap(depth_d.ap()))
            # partition-0 fixup: row -1 := row 0 (edge replication)
            nc.gpsimd.tensor_scalar_add(r3(sem2)[0:1], r4(semn)[0:1, :, 0], 0)
            nc.gpsimd.tensor_scalar_add(r3(m02)[0:1], r4(m0n)[0:1, :, 0], 0.0)
            nc.gpsimd.tensor_scalar_add(r3(d2)[0:1], r4(dn)[0:1, :, 0], 0.0)

        def scr(tag, cols=BPC * 2 * W, dt=f32):
            return pscr.tile([128, cols], dt, tag="scr", name=tag)

        bfields = {}

        def emit_boundary_a():
            # sem equality fields + nb
            eqx = scr("eqx")
            nc.vector.tensor_tensor(blkw(eqx)[:, :, 1:], blkw(semn)[:, :, 1:],
                                    blkw(semn)[:, :, 0:W - 1], op=OP.is_equal)
            nc.gpsimd.memset(blkw(eqx)[:, :, 0:1], 1.0)
            eqy = scr("eqy")
            nc.vector.tensor_tensor(r4(eqy)[:, :, 1], r4(semn)[:, :, 1],
                                    r4(semn)[:, :, 0], op=OP.is_equal)
            nc.vector.tensor_tensor(r4(eqy)[:, :, 0], r4(semn)[:, :, 0],
                                    r3(sem2), op=OP.is_equal)
            nc.gpsimd.tensor_tensor(nb[:], eqx[:], eqy[:], op=OP.mult)

        def emit_boundary_b():
            # mask-0 / depth gradients
            for nat, sh, gx_t, gy_t in ((m0n, m02, "mgx", "mgy"),
                                        (dn, d2, "dgx", "dgy")):
                gx = scr(gx_t)
                nc.gpsimd.tensor_tensor(blkw(gx)[:, :, 1:], blkw(nat)[:, :, 1:],
                                        blkw(nat)[:, :, 0:W - 1],
                                        op=OP.subtract)
                nc.gpsimd.memset(blkw(gx)[:, :, 0:1], 0.0)
                gy = scr(gy_t)
                nc.gpsimd.tensor_tensor(r4(gy)[:, :, 1], r4(nat)[:, :, 1],
                                        r4(nat)[:, :, 0], op=OP.subtract)
                nc.gpsimd.tensor_tensor(r4(gy)[:, :, 0], r4(nat)[:, :, 0],
                                        r3(sh), op=OP.subtract)
                bfields[gx_t] = gx
                bfields[gy_t] = gy

        def emit_boundary_c():
            # instance-boundary indicator ib = (max(mgx^2, mgy^2) > 0.09)
            sqmx = scr("sqmx")
            nc.scalar.activation(sqmx[:], bfields["mgx"][:], AF.Square,
                                 bias=zerob[:, 0:1])
            sqmy = scr("sqmy")
            nc.scalar.activation(sqmy[:], bfields["mgy"][:], AF.Square,
                                 bias=zerob[:, 0:1])
            sqmm = scr("sqmm")
            nc.vector.tensor_tensor(sqmm[:], sqmx[:], sqmy[:], op=OP.max)
            nc.vector.tensor_scalar(ib[:], sqmm[:], 0.09, None, op0=OP.is_gt)

        def emit_boundary_d():
            # depth-boundary terms: s2 = dgx^2 + dgy^2, db = sqrt(s2 + eps)
            sqx = scr("sqx")
            nc.scalar.activation(sqx[:], bfields["dgx"][:], AF.Square,
                                 bias=zerob[:, 0:1])
            sqy = scr("sqy")
            nc.scalar.activation(sqy[:], bfields["dgy"][:], AF.Square,
                                 bias=zerob[:, 0:1])
            nc.gpsimd.tensor_tensor(s2[:], sqx[:], sqy[:], op=OP.add)
            db = scr("db")
            nc.scalar.activation(db[:], s2[:], AF.Sqrt, bias=bias_sq[:, 0:1])
            nbdb = scr("nbdb")
            nc.gpsimd.tensor_tensor(nbdb[:], nb[:], db[:], op=OP.mult)
            nbs2 = scr("nbs2")
            nc.gpsimd.tensor_tensor(nbs2[:], nb[:], s2[:], op=OP.mult)
            nbib = scr("nbib")
            nc.gpsimd.tensor_tensor(nbib[:], nb[:], ib[:], op=OP.mult)
            # per-batch partial sums -> stats cols 6b + {0..4}
            for k, fld in enumerate((ib, nb, nbib, nbdb, nbs2)):
                nc.vector.tensor_reduce(
                    stats[:].rearrange("p (b c) -> p b c", b=BPC)[:, :, k:k + 1],
                    fld[:].rearrange("p (b q) -> p b q", b=BPC),
                    axis=AX.X, op=OP.add)

        # ---------------- emission schedule ----------------
        if do_hist:
            for st in range(ST):
                emit_tile(0, st)
        emit_boundary_dmas()
        emit_boundary_a()
        emit_boundary_b()
        if do_hist:
            emit_tile(1, 0)
            emit_tile(1, 1)
        emit_boundary_c()
        if do_hist:
            emit_tile(1, 2)
            emit_tile(1, 3)
        emit_boundary_d()
        if do_hist:
            emit_entropy_epilogue(0)
            emit_entropy_epilogue(1)

        # ---------------- cross-partition fold + output ----------------
        red_ps = pps.tile([1, NSTAT], f32, tag="red")
        nc.tensor.matmul(red_ps[:], ones[:], stats[:], start=True, stop=True)
        red = pconst.tile([1, NSTAT], f32, tag="redsb")
        nc.scalar.copy(red[:], red_ps[:])
        nc.scalar.dma_start(out_d.ap().rearrange("(a b) -> a b", a=1), red[:])

    nc.compile()
    return nc


def _get_nc():
    global _compiled
    if _compiled is None:
        _compiled = _build()
    return _compiled


def _combine(stats):
    """stats: (NCORES, 12) -> (l_uniform, l_boundary, l_dbc, total) fp32."""
    per_b = stats.reshape(B, 6).astype(np.float64)
    s_ib, s_nb, s_nbib, s_nbdb, s_nbs2, ent = per_b.T
    inter = s_ib - s_nbib
    union = float(N) - s_nb + s_nbib + 1e-8
    l_boundary = 1.0 - np.mean(inter / union)
    l_uniform = ent.sum() / (B * M + 1e-8)
    s_nbv = s_nbdb + 3.0 * s_nbs2          # db^2 == s2 (+1e-24)
    l_dbc = s_nbv.sum() / (B * N)
    total = 0.3 * l_uniform + 0.2 * l_boundary + 0.2 * l_dbc
    return (np.float32(l_uniform), np.float32(l_boundary),
            np.float32(l_dbc), np.float32(total))


def kernel(semantic_pred, instance_masks, depth, spatial_h=H, spatial_w=W):
    global LAST_EXEC_NS, LAST_TRACE
    from concourse.bass_utils import run_bass_kernel_spmd

    sem = np.ascontiguousarray(np.asarray(semantic_pred, dtype=np.int32))
    masks = np.ascontiguousarray(np.asarray(instance_masks, dtype=np.float32))
    dep = np.ascontiguousarray(np.asarray(depth, dtype=np.float32))

    nc = _get_nc()
    in_maps = [
        {"sem": sem[c * BPC:(c + 1) * BPC],
         "masks": masks[c * BPC:(c + 1) * BPC],
         "depth": dep[c * BPC:(c + 1) * BPC]}
        for c in range(NCORES)
    ]
    trace = bool(int(os.environ.get("KERNEL_TRACE", "0")))
    res = run_bass_kernel_spmd(nc, in_maps, list(range(NCORES)), trace=trace)
    LAST_EXEC_NS = res.exec_time_ns
    LAST_TRACE = res.instructions_and_trace
    stats = np.stack([res.results[c]["partials"] for c in range(NCORES)])
    global LAST_STATS
    LAST_STATS = stats
    return _combine(stats)


# revision 21
# speedup vs baseline: 1.2065x; 1.2065x over previous
"""Trainium2 Bass kernel for the ConsistencyLoss problem (v5).

Inputs: semantic_pred (B,N) int32, instance_masks (B,M,N) f32, depth (B,N) f32
with B=16, M=32, N=65536 (H=W=256), C=27 classes. Outputs the scalar tuple
(l_uniform, l_boundary, l_dbc, total).

Sharding: pure data-parallel over batch; 2 batches per core on 8 cores. Each
core emits 12 partial sums (6 per batch); the host combines the 4 scalars.

v5 layout (DMA-saturation rewrite of the v4 baseline):
  * masks stream as ONE [128=(g,m), 4096] f32 HWDGE DMA per supertile (2 MB,
    8 per core, all on the sync ring) instead of 4 quarter DMAs from three
    engines. f32->bf16 casts are split ACT/[gpsimd] at a 64-px-aligned column
    CA so each transpose half depends on exactly one cast.
  * DVE pair-transpose (bf16 pairs bitcast to u32, 32x32 block semantics) as
    before; one-hot built per supertile from a transposed sem tile against a
    materialized iota replica (built once from a [128,54] iota).
  * histogram matmuls: 64 per supertile, stationary [128,64]=(m,e) slices,
    moving [128,54] one-hot; PSUM accumulates the interleaved histogram.
  * boundary/depth losses use a row-pair layout [p, (b, r2, w)] with rows
    (2p, 2p+1): y-gradients for odd rows stay in-tile; even rows use ONE
    shifted [127, (b,w)] HBM DMA per tensor (odd source rows) + a tiny
    partition-0 fixup. All small DMAs ride the scalar ring.
  * 12 partials: per batch {S_ib, S_nb, S_nbib, S_nbdb, S_nbs2, entropy};
    host combines (nbv = nbdb + 3*nbs2 since db^2 = s2).
"""

import os

os.environ.setdefault("MYCRO_LOCAL_CACHE", "1")

import numpy as np
from contextlib import ExitStack

B, M, N, C = 16, 32, 65536, 27
H = W = 256
NCORES = 8
BPC = B // NCORES          # batches per core
ST = 4                     # supertiles per batch (16384 px each)
G = 4                      # partition groups: partition = 32*g + m
FS = 4096                  # pixels per partition per supertile
KB = 64                    # 64-px stationary blocks per supertile
CA = 2560                  # ACT-cast / transpose-A split (64-aligned)
KA = CA // 64              # kb blocks in half A
NSTAT = 12                 # 6 partial sums x 2 batches

LAST_EXEC_NS = None
LAST_TRACE = None
LAST_STATS = None

_compiled = None


def _build():
    import concourse.tile as tile
    from concourse import bacc, mybir

    f32 = mybir.dt.float32
    i32 = mybir.dt.int32
    bf16 = mybir.dt.bfloat16
    OP = mybir.AluOpType
    AX = mybir.AxisListType
    AF = mybir.ActivationFunctionType

    nc = bacc.Bacc("TRN2", target_bir_lowering=False, debug=False,
                   enable_asserts=False, num_swdge_queues=4)
    sem_d = nc.dram_tensor("sem", [BPC, N], i32, kind="ExternalInput")
    masks_d = nc.dram_tensor("masks", [BPC, M, N], f32, kind="ExternalInput")
    depth_d = nc.dram_tensor("depth", [BPC, N], f32, kind="ExternalInput")
    out_d = nc.dram_tensor("partials", [NSTAT], f32, kind="ExternalOutput")

    with tile.TileContext(nc) as tc, ExitStack() as ctx:
        pconst = ctx.enter_context(tc.tile_pool(name="const", bufs=1))
        pin = ctx.enter_context(tc.tile_pool(name="maskin", bufs=3))
        pinb = ctx.enter_context(tc.tile_pool(name="maskinb", bufs=3))
        ptr = ctx.enter_context(tc.tile_pool(name="maskT", bufs=3))
        poh = ctx.enter_context(tc.tile_pool(name="ohp", bufs=2))
        psem = ctx.enter_context(tc.tile_pool(name="semp", bufs=1))
        pbnd = ctx.enter_context(tc.tile_pool(name="bnd", bufs=1))
        pscr = ctx.enter_context(tc.tile_pool(name="scr", bufs=7))
        psm = ctx.enter_context(tc.tile_pool(name="small", bufs=2))
        pps = ctx.enter_context(tc.tile_pool(name="psum", bufs=1, space="PSUM"))

        do_hist = not bool(int(os.environ.get("KERNEL_SKIP_HIST", "0")))

        # ---------------- constants ----------------
        iotac_i = pconst.tile([128, C], i32, tag="iocl")
        nc.gpsimd.iota(iotac_i[:], pattern=[[1, C]], base=0,
                       channel_multiplier=0)
        iotac = pconst.tile([128, C], bf16, tag="iocb")
        nc.scalar.copy(iotac[:], iotac_i[:])
        iorep = pconst.tile([128, 2 * C * KB], bf16, tag="iorep")
        nc.vector.tensor_copy(
            iorep[:].rearrange("p (c u) -> p c u", c=C),
            iotac[:].unsqueeze(2).broadcast_to([128, C, 2 * KB]))

        stats = pconst.tile([128, NSTAT], f32, tag="stats")
        nc.gpsimd.memset(stats[:], 0.0)
        ones = pconst.tile([128, 1], f32, tag="ones")
        nc.gpsimd.memset(ones[:], 1.0)
        bias_ln = pconst.tile([M, 1], f32, tag="bias_ln")
        nc.gpsimd.memset(bias_ln[:], 1e-10)
        bias_sq = pconst.tile([128, 1], f32, tag="bias_sq")
        nc.gpsimd.memset(bias_sq[:], 1e-24)
        zerob = pconst.tile([128, 1], f32, tag="zerob")
        nc.gpsimd.memset(zerob[:], 0.0)
        warm = pconst.tile([128, 1], f32, tag="warm")
        nc.scalar.activation(warm[:], zerob[:], AF.Sqrt, bias=bias_sq[:, 0:1])

        # parity-selection matrices: sele[p, m] = (p == 2m), selo: (p == 2m+1)
        selrow = pconst.tile([64, M], i32, tag="selrow")
        nc.gpsimd.iota(selrow[:], pattern=[[0, M]], base=0, channel_multiplier=1)
        selc2 = pconst.tile([64, M], i32, tag="selc2")
        nc.gpsimd.iota(selc2[:], pattern=[[2, M]], base=0, channel_multiplier=0)
        selc2p = pconst.tile([64, M], i32, tag="selc2p")
        nc.gpsimd.iota(selc2p[:], pattern=[[2, M]], base=1, channel_multiplier=0)
        sele = pconst.tile([64, M], f32, tag="sele")
        nc.vector.tensor_tensor(sele[:], selrow[:], selc2[:], op=OP.is_equal)
        selo = pconst.tile([64, M], f32, tag="selo")
        nc.vector.tensor_tensor(selo[:], selrow[:], selc2p[:], op=OP.is_equal)

        # -------- sem -> transposed one-hot source (both batches) --------
        # s_in[p, st*128 + cu] = sem[st*16384 + p*128 + cu]; after the bf16
        # pair-transpose, sT[32g+p', st*128 + 64k + 2f' + e] is the pixel
        # st*16384 + g*4096 + (2f'+k)*64 + 2p' + e -- i.e. stationary block
        # kb = 2f'+k of supertile st reads one-hot rows from column block
        # (k, f') of sT.
        sT = []
        for b in range(BPC):
            s_in = psem.tile([128, ST * 128], i32, tag=f"s_in{b}")
            nc.sync.dma_start(
                s_in[:],
                sem_d.ap()[b].rearrange("(st p cu) -> p st cu",
                                        st=ST, p=128, cu=128))
            s_bf = psem.tile([128, ST * 128], bf16, tag=f"s_bf{b}")
            nc.scalar.copy(s_bf[:], s_in[:])
            s_t = psem.tile([128, ST * 128], bf16, tag=f"s_t{b}")
            nc.vector.transpose(s_t[:].bitcast(i32), s_bf[:].bitcast(i32))
            sT.append(s_t)

        hist_ps = [pps.tile([64, 2 * C], f32, tag=f"hist{b}", name=f"hist{b}")
                   for b in range(BPC)]

        def emit_tile(b, st):
            tin = pin.tile([128, FS], f32, tag="tin", name="tin")
            nc.sync.dma_start(
                tin[:],
                masks_d.ap()[b].rearrange("m (st g f) -> st g m f",
                                          st=ST, g=G)[st])
            tinb = pinb.tile([128, FS], bf16, tag="tinb", name="tinb")
            nc.scalar.copy(tinb[:, 0:CA], tin[:, 0:CA])
            nc.scalar.copy(tinb[:, CA:FS], tin[:, CA:FS])
            tT = ptr.tile([128, FS], bf16, tag="tT", name="tT")
            nc.vector.transpose(tT[:, 0:CA].bitcast(i32),
                                tinb[:, 0:CA].bitcast(i32))

            # one-hot: m2[p, (c, u)] = (sT[p, st*128 + u] == c); the 128-col
            # sT slice is contiguous, so this is ONE dense 2x-mode TT.
            m2 = poh.tile([128, 2 * C * KB], bf16, tag="m2", name="m2")
            nc.vector.tensor_tensor(
                m2[:].rearrange("p (c u) -> p c u", c=C),
                sT[b][:, st * 128:(st + 1) * 128]
                    .unsqueeze(1).broadcast_to([128, C, 2 * KB]),
                iorep[:].rearrange("p (c u) -> p c u", c=C),
                op=OP.is_equal)

            # u = 64k + 2f + e: stationary kb reads block (k, f) = (kb&1, kb>>1)
            mov = m2[:].rearrange("p (c k f e) -> p k f c e", c=C, k=2, f=32)
            for kb in range(KA):
                nc.tensor.matmul(
                    hist_ps[b][:],
                    tT[:, 64 * kb:64 * (kb + 1)],
                    mov[:, kb & 1, kb >> 1],
                    start=(st == 0 and kb == 0),
                    stop=False)
            nc.vector.transpose(tT[:, CA:FS].bitcast(i32),
                                tinb[:, CA:FS].bitcast(i32))
            for kb in range(KA, KB):
                nc.tensor.matmul(
                    hist_ps[b][:],
                    tT[:, 64 * kb:64 * (kb + 1)],
                    mov[:, kb & 1, kb >> 1],
                    start=False,
                    stop=(st == ST - 1 and kb == KB - 1))

        def emit_entropy_epilogue(b):
            # hist[m, c] = psum[2m, 2c] + psum[2m+1, 2c+1]
            psum_sb = psm.tile([64, 2 * C], f32, tag="psum_sb", name="psum_sb")
            nc.scalar.copy(psum_sb[:], hist_ps[b][:])
            psv = psum_sb[:].rearrange("p (c e) -> p e c", e=2)
            h_ps = pps.tile([M, C], f32, tag=f"hps{b}", name=f"hps{b}")
            nc.tensor.matmul(h_ps[:], sele[:], psv[:, 0], start=True, stop=False)
            nc.tensor.matmul(h_ps[:], selo[:], psv[:, 1], start=False, stop=True)
            hist = psm.tile([M, C], f32, tag="hist_sb", name="hist_sb")
            nc.scalar.copy(hist[:], h_ps[:])
            ms0 = psm.tile([M, 1], f32, tag="ms0", name="ms0")
            nc.vector.tensor_reduce(ms0[:], hist[:], axis=AX.X, op=OP.add)
            ms = psm.tile([M, 1], f32, tag="ms", name="ms")
            nc.vector.tensor_scalar(ms[:], ms0[:], 1e-6, None, op0=OP.add)
            rec = psm.tile([M, 1], f32, tag="rec", name="rec")
            nc.vector.reciprocal(rec[:], ms[:])
            pr = psm.tile([M, C], f32, tag="pr", name="pr")
            nc.vector.tensor_scalar(pr[:], hist[:], rec[:, 0:1], None, op0=OP.mult)
            ql = psm.tile([M, C], f32, tag="ql", name="ql")
            nc.scalar.activation(ql[:], pr[:], AF.Ln, bias=bias_ln[0:M, 0:1])
            escr = psm.tile([M, C], f32, tag="escr", name="escr")
            nc.vector.tensor_tensor(escr[:], pr[:], ql[:], op=OP.mult)
            ent = psm.tile([M, 1], f32, tag="ent", name="ent")
            nc.vector.tensor_reduce(ent[:], escr[:], axis=AX.X, op=OP.add)
            nc.vector.tensor_scalar(stats[0:M, 6 * b + 5:6 * b + 6], ent[:],
                                    -1.0, None, op0=OP.mult)

        # ---------------- boundary tiles ----------------
        # natural layout [p, (b, r2, w)]: rows (2p + r2); shifted tiles hold
        # row 2p-1 (odd source rows) for the even-row y-gradients.
        semn = pbnd.tile([128, BPC * 2 * W], i32, tag="semn", name="semn")
        m0n = pbnd.tile([128, BPC * 2 * W], f32, tag="m0n", name="m0n")
        dn = pbnd.tile([128, BPC * 2 * W], f32, tag="dn", name="dn")
        sem2 = pbnd.tile([128, BPC * W], i32, tag="sem2", name="sem2")
        m02 = pbnd.tile([128, BPC * W], f32, tag="m02", name="m02")
        d2 = pbnd.tile([128, BPC * W], f32, tag="d2", name="d2")
        nb = pbnd.tile([128, BPC * 2 * W], f32, tag="nb", name="nb")
        ib = pbnd.tile([128, BPC * 2 * W], f32, tag="ib", name="ib")
        s2 = pbnd.tile([128, BPC * 2 * W], f32, tag="s2", name="s2")

        m0_src = masks_d.ap().rearrange("b m n -> m b n")[0]

        def nat_ap(t):
            return t.rearrange("b (p r2 w) -> p b r2 w", p=128, r2=2, w=W)

        def odd_ap(t):
            # odd source rows 1,3,...,253 -> partitions 1..127
            return t[:, W:W + 127 * 2 * W].rearrange(
                "b (p v) -> p b v", p=127)[:, :, 0:W]

        def r4(t):
            return t[:].rearrange("p (b r2 w) -> p b r2 w", b=BPC, r2=2)

        def r3(t):
            return t[:].rearrange("p (b w) -> p b w", b=BPC)

        def blkw(t):
            return t[:].rearrange("p (q w) -> p q w", w=W)

        def emit_boundary_dmas():
            nc.gpsimd.dma_start(r4(semn), nat_ap(sem_d.ap()))
            nc.gpsimd.dma_start(r4(m0n), nat_ap(m0_src))
            nc.gpsimd.dma_start(r4(dn), nat_ap(depth_d.ap()))
            nc.gpsimd.dma_start(r3(sem2)[1:128], odd_ap(sem_d.ap()))
            nc.gpsimd.dma_start(r3(m02)[1:128], odd_ap(m0_src))
            nc.gpsimd.dma_start(r3(d2)[1:128], odd_ap(depth_d.ap()))
            # partition-0 fixup: row -1 := row 0 (edge replication)
            nc.gpsimd.tensor_scalar_add(r3(sem2)[0:1], r4(semn)[0:1, :, 0], 0)
            nc.gpsimd.tensor_scalar_add(r3(m02)[0:1], r4(m0n)[0:1, :, 0], 0.0)
            nc.gpsimd.tensor_scalar_add(r3(d2)[0:1], r4(dn)[0:1, :, 0], 0.0)

        def scr(tag, cols=BPC * 2 * W, dt=f32):
            return pscr.tile([128, cols], dt, tag="scr", name=tag)

        bfields = {}

        def emit_boundary_a():
            # sem equality fields + nb
            eqx = scr("eqx")
            nc.vector.tensor_tensor(blkw(eqx)[:, :, 1:], blkw(semn)[:, :, 1:],
                                    blkw(semn)[:, :, 0:W - 1], op=OP.is_equal)
            nc.gpsimd.memset(blkw(eqx)[:, :, 0:1], 1.0)
            eqy = scr("eqy")
            nc.vector.tensor_tensor(r4(eqy)[:, :, 1], r4(semn)[:, :, 1],
                                    r4(semn)[:, :, 0], op=OP.is_equal)
            nc.vector.tensor_tensor(r4(eqy)[:, :, 0], r4(semn)[:, :, 0],
                                    r3(sem2), op=OP.is_equal)
            nc.gpsimd.tensor_tensor(nb[:], eqx[:], eqy[:], op=OP.mult)

        def emit_boundary_b():
            # mask-0 / depth gradients
            for nat, sh, gx_t, gy_t in ((m0n, m02, "mgx", "mgy"),
                                        (dn, d2, "dgx", "dgy")):
                gx = scr(gx_t)
                nc.gpsimd.tensor_tensor(blkw(gx)[:, :, 1:], blkw(nat)[:, :, 1:],
                                        blkw(nat)[:, :, 0:W - 1],
                                        op=OP.subtract)
                nc.gpsimd.memset(blkw(gx)[:, :, 0:1], 0.0)
                gy = scr(gy_t)
                nc.gpsimd.tensor_tensor(r4(gy)[:, :, 1], r4(nat)[:, :, 1],
                                        r4(nat)[:, :, 0], op=OP.subtract)
                nc.gpsimd.tensor_tensor(r4(gy)[:, :, 0], r4(nat)[:, :, 0],
                                        r3(sh), op=OP.subtract)
                bfields[gx_t] = gx
                bfields[gy_t] = gy

        def emit_boundary_c():
            # instance-boundary indicator ib = (max(mgx^2, mgy^2) > 0.09)
            sqmx = scr("sqmx")
            nc.gpsimd.tensor_tensor(sqmx[:], bfields["mgx"][:],
                                    bfields["mgx"][:], op=OP.mult)
            sqmy = scr("sqmy")
            nc.gpsimd.tensor_tensor(sqmy[:], bfields["mgy"][:],
                                    bfields["mgy"][:], op=OP.mult)
            sqmm = scr("sqmm")
            nc.vector.tensor_tensor(sqmm[:], sqmx[:], sqmy[:], op=OP.max)
            nc.vector.tensor_scalar(ib[:], sqmm[:], 0.09, None, op0=OP.is_gt)

        def emit_boundary_d():
            # depth-boundary terms: s2 = dgx^2 + dgy^2, db = sqrt(s2 + eps)
            sqx = scr("sqx")
            nc.gpsimd.tensor_tensor(sqx[:], bfields["dgx"][:],
                                    bfields["dgx"][:], op=OP.mult)
            sqy = scr("sqy")
            nc.gpsimd.tensor_tensor(sqy[:], bfields["dgy"][:],
                                    bfields["dgy"][:], op=OP.mult)
            nc.gpsimd.tensor_tensor(s2[:], sqx[:], sqy[:], op=OP.add)
            db = scr("db")
            nc.scalar.activation(db[:], s2[:], AF.Sqrt, bias=bias_sq[:, 0:1])
            nbdb = scr("nbdb")
            nc.gpsimd.tensor_tensor(nbdb[:], nb[:], db[:], op=OP.mult)
            nbs2 = scr("nbs2")
            nc.gpsimd.tensor_tensor(nbs2[:], nb[:], s2[:], op=OP.mult)
            nbib = scr("nbib")
            nc.gpsimd.tensor_tensor(nbib[:], nb[:], ib[:], op=OP.mult)
            # per-batch partial sums -> stats cols 6b + {0..4}
            for k, fld in enumerate((ib, nb, nbib, nbdb, nbs2)):
                nc.vector.tensor_reduce(
                    stats[:].rearrange("p (b c) -> p b c", b=BPC)[:, :, k:k + 1],
                    fld[:].rearrange("p (b q) -> p b q", b=BPC),
                    axis=AX.X, op=OP.add)

        # ---------------- emission schedule ----------------
        if do_hist:
            for st in range(ST):
                emit_tile(0, st)
        emit_boundary_dmas()
        emit_boundary_a()
        emit_boundary_b()
        if do_hist:
            emit_tile(1, 0)
            emit_tile(1, 1)
        emit_boundary_c()
        if do_hist:
            emit_tile(1, 2)
            emit_tile(1, 3)
        emit_boundary_d()
        if do_hist:
            emit_entropy_epilogue(0)
            emit_entropy_epilogue(1)

        # ---------------- cross-partition fold + output ----------------
        red_ps = pps.tile([1, NSTAT], f32, tag="red")
        nc.tensor.matmul(red_ps[:], ones[:], stats[:], start=True, stop=True)
        red = pconst.tile([1, NSTAT], f32, tag="redsb")
        nc.scalar.copy(red[:], red_ps[:])
        nc.scalar.dma_start(out_d.ap().rearrange("(a b) -> a b", a=1), red[:])

    nc.compile()
    return nc


def _get_nc():
    global _compiled
    if _compiled is None:
        _compiled = _build()
    return _compiled


def _combine(stats):
    """stats: (NCORES, 12) -> (l_uniform, l_boundary, l_dbc, total) fp32."""
    per_b = stats.reshape(B, 6).astype(np.float64)
    s_ib, s_nb, s_nbib, s_nbdb, s_nbs2, ent = per_b.T
    inter = s_ib - s_nbib
    union = float(N) - s_nb + s_nbib + 1e-8
    l_boundary = 1.0 - np.mean(inter / union)
    l_uniform = ent.sum() / (B * M + 1e-8)
    s_nbv = s_nbdb + 3.0 * s_nbs2          # db^2 == s2 (+1e-24)
    l_dbc = s_nbv.sum() / (B * N)
    total = 0.3 * l_uniform + 0.2 * l_boundary + 0.2 * l_dbc
    return (np.float32(l_uniform), np.float32(l_boundary),
            np.float32(l_dbc), np.float32(total))


def kernel(semantic_pred, instance_masks, depth, spatial_h=H, spatial_w=W):
    global LAST_EXEC_NS, LAST_TRACE
    from concourse.bass_utils import run_bass_kernel_spmd

    sem = np.ascontiguousarray(np.asarray(semantic_pred, dtype=np.int32))
    masks = np.ascontiguousarray(np.asarray(instance_masks, dtype=np.float32))
    dep = np.ascontiguousarray(np.asarray(depth, dtype=np.float32))

    nc = _get_nc()
    in_maps = [
        {"sem": sem[c * BPC:(c + 1) * BPC],
         "masks": masks[c * BPC:(c + 1) * BPC],
         "depth": dep[c * BPC:(c + 1) * BPC]}
        for c in range(NCORES)
    ]
    trace = bool(int(os.environ.get("KERNEL_TRACE", "0")))
    res = run_bass_kernel_spmd(nc, in_maps, list(range(NCORES)), trace=trace)
    LAST_EXEC_NS = res.exec_time_ns
    LAST_TRACE = res.instructions_and_trace
    stats = np.stack([res.results[c]["partials"] for c in range(NCORES)])
    global LAST_STATS
    LAST_STATS = stats
    return _combine(stats)


# revision 23
# speedup vs baseline: 2.1308x; 1.7661x over previous
"""Trainium2 Bass kernel for the ConsistencyLoss problem (v5).

Inputs: semantic_pred (B,N) int32, instance_masks (B,M,N) f32, depth (B,N) f32
with B=16, M=32, N=65536 (H=W=256), C=27 classes. Outputs the scalar tuple
(l_uniform, l_boundary, l_dbc, total).

Sharding: pure data-parallel over batch; 2 batches per core on 8 cores. Each
core emits 12 partial sums (6 per batch); the host combines the 4 scalars.

v5 layout (DMA-saturation rewrite of the v4 baseline):
  * masks stream as ONE [128=(g,m), 4096] f32 HWDGE DMA per supertile (2 MB,
    8 per core, all on the sync ring) instead of 4 quarter DMAs from three
    engines. f32->bf16 casts are split ACT/[gpsimd] at a 64-px-aligned column
    CA so each transpose half depends on exactly one cast.
  * DVE pair-transpose (bf16 pairs bitcast to u32, 32x32 block semantics) as
    before; one-hot built per supertile from a transposed sem tile against a
    materialized iota replica (built once from a [128,54] iota).
  * histogram matmuls: 64 per supertile, stationary [128,64]=(m,e) slices,
    moving [128,54] one-hot; PSUM accumulates the interleaved histogram.
  * boundary/depth losses use a row-pair layout [p, (b, r2, w)] with rows
    (2p, 2p+1): y-gradients for odd rows stay in-tile; even rows use ONE
    shifted [127, (b,w)] HBM DMA per tensor (odd source rows) + a tiny
    partition-0 fixup. All small DMAs ride the scalar ring.
  * 12 partials: per batch {S_ib, S_nb, S_nbib, S_nbdb, S_nbs2, entropy};
    host combines (nbv = nbdb + 3*nbs2 since db^2 = s2).
"""

import os

os.environ.setdefault("MYCRO_LOCAL_CACHE", "1")

import numpy as np
from contextlib import ExitStack

B, M, N, C = 16, 32, 65536, 27
H = W = 256
NCORES = 8
BPC = B // NCORES          # batches per core
ST = 4                     # supertiles per batch (16384 px each)
G = 4                      # partition groups: partition = 32*g + m
FS = 4096                  # pixels per partition per supertile
KB = 64                    # 64-px stationary blocks per supertile
CA = 2560                  # ACT-cast / transpose-A split (64-aligned)
KA = CA // 64              # kb blocks in half A
NSTAT = 12                 # 6 partial sums x 2 batches

LAST_EXEC_NS = None
LAST_TRACE = None
LAST_STATS = None

_compiled = None


def _build():
    import concourse.tile as tile
    from concourse import bacc, mybir

    f32 = mybir.dt.float32
    i32 = mybir.dt.int32
    bf16 = mybir.dt.bfloat16
    OP = mybir.AluOpType
    AX = mybir.AxisListType
    AF = mybir.ActivationFunctionType

    nc = bacc.Bacc("TRN2", target_bir_lowering=False, debug=False,
                   enable_asserts=False, num_swdge_queues=4)
    sem_d = nc.dram_tensor("sem", [BPC, N], i32, kind="ExternalInput")
    masks_d = nc.dram_tensor("masks", [BPC, M, N], f32, kind="ExternalInput")
    depth_d = nc.dram_tensor("depth", [BPC, N], f32, kind="ExternalInput")
    out_d = nc.dram_tensor("partials", [NSTAT], f32, kind="ExternalOutput")

    with tile.TileContext(nc) as tc, ExitStack() as ctx:
        pconst = ctx.enter_context(tc.tile_pool(name="const", bufs=1))
        pin = ctx.enter_context(tc.tile_pool(name="maskin", bufs=3))
        pinb = ctx.enter_context(tc.tile_pool(name="maskinb", bufs=3))
        ptr = ctx.enter_context(tc.tile_pool(name="maskT", bufs=3))
        poh = ctx.enter_context(tc.tile_pool(name="ohp", bufs=2))
        psem = ctx.enter_context(tc.tile_pool(name="semp", bufs=1))
        pbnd = ctx.enter_context(tc.tile_pool(name="bnd", bufs=1))
        pscr = ctx.enter_context(tc.tile_pool(name="scr", bufs=7))
        psm = ctx.enter_context(tc.tile_pool(name="small", bufs=2))
        pps = ctx.enter_context(tc.tile_pool(name="psum", bufs=1, space="PSUM"))

        do_hist = not bool(int(os.environ.get("KERNEL_SKIP_HIST", "0")))

        # ---------------- constants ----------------
        iotac_i = pconst.tile([128, C], i32, tag="iocl")
        nc.gpsimd.iota(iotac_i[:], pattern=[[1, C]], base=0,
                       channel_multiplier=0)
        iotac = pconst.tile([128, C], bf16, tag="iocb")
        nc.scalar.copy(iotac[:], iotac_i[:])
        iorep = pconst.tile([128, 2 * C * KB], bf16, tag="iorep")
        nc.vector.tensor_copy(
            iorep[:].rearrange("p (c u) -> p c u", c=C),
            iotac[:].unsqueeze(2).broadcast_to([128, C, 2 * KB]))

        stats = pconst.tile([128, NSTAT], f32, tag="stats")
        nc.gpsimd.memset(stats[:], 0.0)
        ones = pconst.tile([128, 1], f32, tag="ones")
        nc.gpsimd.memset(ones[:], 1.0)
        bias_ln = pconst.tile([M, 1], f32, tag="bias_ln")
        nc.gpsimd.memset(bias_ln[:], 1e-10)
        bias_sq = pconst.tile([128, 1], f32, tag="bias_sq")
        nc.gpsimd.memset(bias_sq[:], 1e-24)
        zerob = pconst.tile([128, 1], f32, tag="zerob")
        nc.gpsimd.memset(zerob[:], 0.0)
        warm = pconst.tile([128, 1], f32, tag="warm")
        nc.scalar.activation(warm[:], zerob[:], AF.Sqrt, bias=bias_sq[:, 0:1])

        # parity-selection matrices: sele[p, m] = (p == 2m), selo: (p == 2m+1)
        selrow = pconst.tile([64, M], i32, tag="selrow")
        nc.gpsimd.iota(selrow[:], pattern=[[0, M]], base=0, channel_multiplier=1)
        selc2 = pconst.tile([64, M], i32, tag="selc2")
        nc.gpsimd.iota(selc2[:], pattern=[[2, M]], base=0, channel_multiplier=0)
        selc2p = pconst.tile([64, M], i32, tag="selc2p")
        nc.gpsimd.iota(selc2p[:], pattern=[[2, M]], base=1, channel_multiplier=0)
        sele = pconst.tile([64, M], f32, tag="sele")
        nc.vector.tensor_tensor(sele[:], selrow[:], selc2[:], op=OP.is_equal)
        selo = pconst.tile([64, M], f32, tag="selo")
        nc.vector.tensor_tensor(selo[:], selrow[:], selc2p[:], op=OP.is_equal)

        # -------- sem -> transposed one-hot source (both batches) --------
        # s_in[p, st*128 + cu] = sem[st*16384 + p*128 + cu]; after the bf16
        # pair-transpose, sT[32g+p', st*128 + 64k + 2f' + e] is the pixel
        # st*16384 + g*4096 + (2f'+k)*64 + 2p' + e -- i.e. stationary block
        # kb = 2f'+k of supertile st reads one-hot rows from column block
        # (k, f') of sT.
        sT = []
        for b in range(BPC):
            s_in = psem.tile([128, ST * 128], i32, tag=f"s_in{b}")
            nc.sync.dma_start(
                s_in[:],
                sem_d.ap()[b].rearrange("(st p cu) -> p st cu",
                                        st=ST, p=128, cu=128))
            s_bf = psem.tile([128, ST * 128], bf16, tag=f"s_bf{b}")
            nc.scalar.copy(s_bf[:], s_in[:])
            s_t = psem.tile([128, ST * 128], bf16, tag=f"s_t{b}")
            nc.vector.transpose(s_t[:].bitcast(i32), s_bf[:].bitcast(i32))
            sT.append(s_t)

        hist_ps = [pps.tile([64, 2 * C], f32, tag=f"hist{b}", name=f"hist{b}")
                   for b in range(BPC)]

        def emit_tile(b, st):
            tin = pin.tile([128, FS], f32, tag="tin", name="tin")
            src = masks_d.ap()[b].rearrange("m (st g f) -> st g m f",
                                            st=ST, g=G)
            for g in range(G):
                nc.sync.dma_start(tin[32 * g:32 * (g + 1), :], src[st, g])
            tinb = pinb.tile([128, FS], bf16, tag="tinb", name="tinb")
            nc.scalar.copy(tinb[:, 0:CA], tin[:, 0:CA])
            nc.scalar.copy(tinb[:, CA:FS], tin[:, CA:FS])
            tT = ptr.tile([128, FS], bf16, tag="tT", name="tT")
            nc.vector.transpose(tT[:, 0:CA].bitcast(i32),
                                tinb[:, 0:CA].bitcast(i32))

            # one-hot: m2[p, (c, u)] = (sT[p, st*128 + u] == c); the 128-col
            # sT slice is contiguous, so this is ONE dense 2x-mode TT.
            m2 = poh.tile([128, 2 * C * KB], bf16, tag="m2", name="m2")
            nc.vector.tensor_tensor(
                m2[:].rearrange("p (c u) -> p c u", c=C),
                sT[b][:, st * 128:(st + 1) * 128]
                    .unsqueeze(1).broadcast_to([128, C, 2 * KB]),
                iorep[:].rearrange("p (c u) -> p c u", c=C),
                op=OP.is_equal)

            # u = 64k + 2f + e: stationary kb reads block (k, f) = (kb&1, kb>>1)
            mov = m2[:].rearrange("p (c k f e) -> p k f c e", c=C, k=2, f=32)
            for kb in range(KA):
                nc.tensor.matmul(
                    hist_ps[b][:],
                    tT[:, 64 * kb:64 * (kb + 1)],
                    mov[:, kb & 1, kb >> 1],
                    start=(st == 0 and kb == 0),
                    stop=False)
            nc.vector.transpose(tT[:, CA:FS].bitcast(i32),
                                tinb[:, CA:FS].bitcast(i32))
            for kb in range(KA, KB):
                nc.tensor.matmul(
                    hist_ps[b][:],
                    tT[:, 64 * kb:64 * (kb + 1)],
                    mov[:, kb & 1, kb >> 1],
                    start=False,
                    stop=(st == ST - 1 and kb == KB - 1))

        def emit_entropy_epilogue(b):
            # hist[m, c] = psum[2m, 2c] + psum[2m+1, 2c+1]
            psum_sb = psm.tile([64, 2 * C], f32, tag="psum_sb", name="psum_sb")
            nc.scalar.copy(psum_sb[:], hist_ps[b][:])
            psv = psum_sb[:].rearrange("p (c e) -> p e c", e=2)
            h_ps = pps.tile([M, C], f32, tag=f"hps{b}", name=f"hps{b}")
            nc.tensor.matmul(h_ps[:], sele[:], psv[:, 0], start=True, stop=False)
            nc.tensor.matmul(h_ps[:], selo[:], psv[:, 1], start=False, stop=True)
            hist = psm.tile([M, C], f32, tag="hist_sb", name="hist_sb")
            nc.scalar.copy(hist[:], h_ps[:])
            ms0 = psm.tile([M, 1], f32, tag="ms0", name="ms0")
            nc.vector.tensor_reduce(ms0[:], hist[:], axis=AX.X, op=OP.add)
            ms = psm.tile([M, 1], f32, tag="ms", name="ms")
            nc.vector.tensor_scalar(ms[:], ms0[:], 1e-6, None, op0=OP.add)
            rec = psm.tile([M, 1], f32, tag="rec", name="rec")
            nc.vector.reciprocal(rec[:], ms[:])
            pr = psm.tile([M, C], f32, tag="pr", name="pr")
            nc.vector.tensor_scalar(pr[:], hist[:], rec[:, 0:1], None, op0=OP.mult)
            ql = psm.tile([M, C], f32, tag="ql", name="ql")
            nc.scalar.activation(ql[:], pr[:], AF.Ln, bias=bias_ln[0:M, 0:1])
            escr = psm.tile([M, C], f32, tag="escr", name="escr")
            nc.vector.tensor_tensor(escr[:], pr[:], ql[:], op=OP.mult)
            ent = psm.tile([M, 1], f32, tag="ent", name="ent")
            nc.vector.tensor_reduce(ent[:], escr[:], axis=AX.X, op=OP.add)
            nc.vector.tensor_scalar(stats[0:M, 6 * b + 5:6 * b + 6], ent[:],
                                    -1.0, None, op0=OP.mult)

        # ---------------- boundary tiles ----------------
        # natural layout [p, (b, r2, w)]: rows (2p + r2); shifted tiles hold
        # row 2p-1 (odd source rows) for the even-row y-gradients.
        semn = pbnd.tile([128, BPC * 2 * W], i32, tag="semn", name="semn")
        m0n = pbnd.tile([128, BPC * 2 * W], f32, tag="m0n", name="m0n")
        dn = pbnd.tile([128, BPC * 2 * W], f32, tag="dn", name="dn")
        sem2 = pbnd.tile([128, BPC * W], i32, tag="sem2", name="sem2")
        m02 = pbnd.tile([128, BPC * W], f32, tag="m02", name="m02")
        d2 = pbnd.tile([128, BPC * W], f32, tag="d2", name="d2")
        nb = pbnd.tile([128, BPC * 2 * W], f32, tag="nb", name="nb")
        ib = pbnd.tile([128, BPC * 2 * W], f32, tag="ib", name="ib")
        s2 = pbnd.tile([128, BPC * 2 * W], f32, tag="s2", name="s2")

        m0_src = masks_d.ap().rearrange("b m n -> m b n")[0]

        def nat_ap(t):
            return t.rearrange("b (p r2 w) -> p b r2 w", p=128, r2=2, w=W)

        def odd_ap(t):
            # odd source rows 1,3,...,253 -> partitions 1..127
            return t[:, W:W + 127 * 2 * W].rearrange(
                "b (p v) -> p b v", p=127)[:, :, 0:W]

        def r4(t):
            return t[:].rearrange("p (b r2 w) -> p b r2 w", b=BPC, r2=2)

        def r3(t):
            return t[:].rearrange("p (b w) -> p b w", b=BPC)

        def blkw(t):
            return t[:].rearrange("p (q w) -> p q w", w=W)

        def emit_boundary_dmas():
            nc.gpsimd.dma_start(r4(semn), nat_ap(sem_d.ap()))
            nc.gpsimd.dma_start(r4(m0n), nat_ap(m0_src))
            nc.gpsimd.dma_start(r4(dn), nat_ap(depth_d.ap()))
            nc.gpsimd.dma_start(r3(sem2)[1:128], odd_ap(sem_d.ap()))
            nc.gpsimd.dma_start(r3(m02)[1:128], odd_ap(m0_src))
            nc.gpsimd.dma_start(r3(d2)[1:128], odd_ap(depth_d.ap()))
            # partition-0 fixup: row -1 := row 0 (edge replication)
            nc.vector.tensor_copy(r3(sem2)[0:1], r4(semn)[0:1, :, 0])
            nc.vector.tensor_copy(r3(m02)[0:1], r4(m0n)[0:1, :, 0])
            nc.vector.tensor_copy(r3(d2)[0:1], r4(dn)[0:1, :, 0])

        def scr(tag, cols=BPC * 2 * W, dt=f32):
            return pscr.tile([128, cols], dt, tag="scr", name=tag)

        bfields = {}

        def emit_boundary_a():
            # sem equality fields + nb
            eqx = scr("eqx")
            nc.vector.tensor_tensor(blkw(eqx)[:, :, 1:], blkw(semn)[:, :, 1:],
                                    blkw(semn)[:, :, 0:W - 1], op=OP.is_equal)
            nc.gpsimd.memset(blkw(eqx)[:, :, 0:1], 1.0)
            eqy = scr("eqy")
            nc.vector.tensor_tensor(r4(eqy)[:, :, 1], r4(semn)[:, :, 1],
                                    r4(semn)[:, :, 0], op=OP.is_equal)
            nc.vector.tensor_tensor(r4(eqy)[:, :, 0], r4(semn)[:, :, 0],
                                    r3(sem2), op=OP.is_equal)
            nc.gpsimd.tensor_tensor(nb[:], eqx[:], eqy[:], op=OP.mult)

        def emit_boundary_b():
            # mask-0 / depth gradients
            for nat, sh, gx_t, gy_t in ((m0n, m02, "mgx", "mgy"),
                                        (dn, d2, "dgx", "dgy")):
                gx = scr(gx_t)
                nc.gpsimd.tensor_tensor(blkw(gx)[:, :, 1:], blkw(nat)[:, :, 1:],
                                        blkw(nat)[:, :, 0:W - 1],
                                        op=OP.subtract)
                nc.gpsimd.memset(blkw(gx)[:, :, 0:1], 0.0)
                gy = scr(gy_t)
                nc.gpsimd.tensor_tensor(r4(gy)[:, :, 1], r4(nat)[:, :, 1],
                                        r4(nat)[:, :, 0], op=OP.subtract)
                nc.gpsimd.tensor_tensor(r4(gy)[:, :, 0], r4(nat)[:, :, 0],
                                        r3(sh), op=OP.subtract)
                bfields[gx_t] = gx
                bfields[gy_t] = gy

        def emit_boundary_c():
            # instance-boundary indicator ib = (max(mgx^2, mgy^2) > 0.09)
            sqmx = scr("sqmx")
            nc.gpsimd.tensor_tensor(sqmx[:], bfields["mgx"][:],
                                    bfields["mgx"][:], op=OP.mult)
            sqmy = scr("sqmy")
            nc.gpsimd.tensor_tensor(sqmy[:], bfields["mgy"][:],
                                    bfields["mgy"][:], op=OP.mult)
            sqmm = scr("sqmm")
            nc.vector.tensor_tensor(sqmm[:], sqmx[:], sqmy[:], op=OP.max)
            nc.vector.tensor_scalar(ib[:], sqmm[:], 0.09, None, op0=OP.is_gt)

        def emit_boundary_d():
            # depth-boundary terms: s2 = dgx^2 + dgy^2, db = sqrt(s2 + eps)
            sqx = scr("sqx")
            nc.gpsimd.tensor_tensor(sqx[:], bfields["dgx"][:],
                                    bfields["dgx"][:], op=OP.mult)
            sqy = scr("sqy")
            nc.gpsimd.tensor_tensor(sqy[:], bfields["dgy"][:],
                                    bfields["dgy"][:], op=OP.mult)
            nc.gpsimd.tensor_tensor(s2[:], sqx[:], sqy[:], op=OP.add)
            db = scr("db")
            nc.scalar.activation(db[:], s2[:], AF.Sqrt, bias=bias_sq[:, 0:1])
            nbdb = scr("nbdb")
            nc.gpsimd.tensor_tensor(nbdb[:], nb[:], db[:], op=OP.mult)
            nbs2 = scr("nbs2")
            nc.gpsimd.tensor_tensor(nbs2[:], nb[:], s2[:], op=OP.mult)
            nbib = scr("nbib")
            nc.gpsimd.tensor_tensor(nbib[:], nb[:], ib[:], op=OP.mult)
            # per-batch partial sums -> stats cols 6b + {0..4}
            for k, fld in enumerate((ib, nb, nbib, nbdb, nbs2)):
                nc.vector.tensor_reduce(
                    stats[:].rearrange("p (b c) -> p b c", b=BPC)[:, :, k:k + 1],
                    fld[:].rearrange("p (b q) -> p b q", b=BPC),
                    axis=AX.X, op=OP.add)

        # ---------------- emission schedule ----------------
        if do_hist:
            for st in range(ST):
                emit_tile(0, st)
        emit_boundary_dmas()
        emit_boundary_a()
        emit_boundary_b()
        if do_hist:
            emit_tile(1, 0)
            emit_tile(1, 1)
        emit_boundary_c()
        if do_hist:
            emit_tile(1, 2)
            emit_tile(1, 3)
        emit_boundary_d()
        if do_hist:
            emit_entropy_epilogue(0)
            emit_entropy_epilogue(1)

        # ---------------- cross-partition fold + output ----------------
        red_ps = pps.tile([1, NSTAT], f32, tag="red")
        nc.tensor.matmul(red_ps[:], ones[:], stats[:], start=True, stop=True)
        red = pconst.tile([1, NSTAT], f32, tag="redsb")
        nc.scalar.copy(red[:], red_ps[:])
        nc.scalar.dma_start(out_d.ap().rearrange("(a b) -> a b", a=1), red[:])

    nc.compile()
    return nc


def _get_nc():
    global _compiled
    if _compiled is None:
        _compiled = _build()
    return _compiled


def _combine(stats):
    """stats: (NCORES, 12) -> (l_uniform, l_boundary, l_dbc, total) fp32."""
    per_b = stats.reshape(B, 6).astype(np.float64)
    s_ib, s_nb, s_nbib, s_nbdb, s_nbs2, ent = per_b.T
    inter = s_ib - s_nbib
    union = float(N) - s_nb + s_nbib + 1e-8
    l_boundary = 1.0 - np.mean(inter / union)
    l_uniform = ent.sum() / (B * M + 1e-8)
    s_nbv = s_nbdb + 3.0 * s_nbs2          # db^2 == s2 (+1e-24)
    l_dbc = s_nbv.sum() / (B * N)
    total = 0.3 * l_uniform + 0.2 * l_boundary + 0.2 * l_dbc
    return (np.float32(l_uniform), np.float32(l_boundary),
            np.float32(l_dbc), np.float32(total))


def kernel(semantic_pred, instance_masks, depth, spatial_h=H, spatial_w=W):
    global LAST_EXEC_NS, LAST_TRACE
    from concourse.bass_utils import run_bass_kernel_spmd

    sem = np.ascontiguousarray(np.asarray(semantic_pred, dtype=np.int32))
    masks = np.ascontiguousarray(np.asarray(instance_masks, dtype=np.float32))
    dep = np.ascontiguousarray(np.asarray(depth, dtype=np.float32))

    nc = _get_nc()
    in_maps = [
        {"sem": sem[c * BPC:(c + 1) * BPC],
         "masks": masks[c * BPC:(c + 1) * BPC],
         "depth": dep[c * BPC:(c + 1) * BPC]}
        for c in range(NCORES)
    ]
    trace = bool(int(os.environ.get("KERNEL_TRACE", "0")))
    res = run_bass_kernel_spmd(nc, in_maps, list(range(NCORES)), trace=trace)
    LAST_EXEC_NS = res.exec_time_ns
    LAST_TRACE = res.instructions_and_trace
    stats = np.stack([res.results[c]["partials"] for c in range(NCORES)])
    global LAST_STATS
    LAST_STATS = stats
    return _combine(stats)
